# revision 18
# baseline (speedup 1.0000x reference)
"""Trainium2 Bass kernel for nn_CrossAttention (non-softmax bilinear attention).

Reference computation (B=2, C=256, C8=32, D=16, H=64, W=64):
    q = conv1x1(x1, Wq) + bq           [B,C8,D,H,W]
    k = conv1x1(x2, Wk) + bk           [B,C8,D,H,W]
    v = conv1x1(x2, Wv) + bv           [B,C ,D,H,W]
    attn[b,d,h,x,y] = sum_c q[b,c,d,h,x] k[b,c,d,h,y]
    out[b,c,d,h,x]  = sum_y attn[b,d,h,x,y] v[b,c,d,h,y]

Sharding: the (b,d) axis (32 slices) is split across 8 cores, 4 slices each.
Each core processes its slices in hw-blocks of 2048 (32 h rows = 16 h-pairs,
8 two-pair "couples"):
  - All matmuls run in float32r (1 cycle/row at N>=256, vs 4 for plain fp32).
    f32r constraints: producers of matmul inputs are DMAs, DVE copies/adds, or
    ACT Identity ops writing f32r-tagged tiles; N>=256; tile_position (0,0).
  - q,k are [32, 2048] tiles (channels on partitions 0:32), bias added by the
    scalar engine during PSUM eviction.
  - v is produced directly transposed (vT[y,c], +bv via a broadcast tile) by
    using the x2 block as the stationary operand; PSUM evictions are batched
    per couple ([128,512]) to halve DVE/ACT op count.
  - attention per h-pair as a [128,256] cross block over the couple's
    x-window; h-cross terms are zeroed on SBUF (3 persistent attn tiles whose
    off-diagonal quadrants are zeroed once and never rewritten).
  - out is computed transposed: outT[x,c] = at[y,x]^T @ vT[y,c], one N=256
    matmul per pair accumulating into a per-couple [128,512] PSUM tile; the
    output DRAM tensor is [S, HW, C] and the host transposes back.
"""

import sys
import numpy as np

if "/opt/trn_rl_repo" not in sys.path:
    sys.path.insert(0, "/opt/trn_rl_repo")

import concourse.bass as bass
import concourse.bacc as bacc
import concourse.mybir as mybir
from concourse import tile
from concourse.bass_utils import run_bass_kernel_spmd

F32 = mybir.dt.float32
F32R = mybir.dt.float32r

B, C, D, H, W = 2, 256, 16, 64, 64
C8 = C // 8
NCORES = 8
S = (B * D) // NCORES          # (b,d) slices per core
HWF = H * W                    # 4096
BLK = 2048                     # hw columns per block
NBLK = HWF // BLK
NPAIR = BLK // 128             # 16 h-pairs per block


def build_nc() -> bass.Bass:
    # Bacc (vs plain Bass) is required: its compile() pipeline runs
    # move_matmul_waits_to_ldweights + generate_event_semaphores, which
    # split multi-sem waits that the TRN2 Matmult encoding can't carry.
    nc = bacc.Bacc()

    x1c = nc.declare_dram_parameter("x1c", [S, C, HWF], F32R, isOutput=False)
    x2c = nc.declare_dram_parameter("x2c", [S, C, HWF], F32R, isOutput=False)
    wqt = nc.declare_dram_parameter("wqt", [C, C8], F32R, isOutput=False)
    wkt = nc.declare_dram_parameter("wkt", [C, C8], F32R, isOutput=False)
    wvt = nc.declare_dram_parameter("wvt", [C, C], F32R, isOutput=False)
    bq32 = nc.declare_dram_parameter("bq32", [C8, 1], F32, isOutput=False)
    bk32 = nc.declare_dram_parameter("bk32", [C8, 1], F32, isOutput=False)
    bvr2 = nc.declare_dram_parameter("bvr2", [1, 2 * C], F32, isOutput=False)
    atz = nc.declare_dram_parameter("atz", [128, 128], F32R, isOutput=False)
    outc = nc.declare_dram_parameter("outc", [S, HWF, C], F32, isOutput=True)

    with tile.TileContext(nc) as tc:
        with (
            tc.tile_pool(name="const", bufs=1) as const,
            tc.tile_pool(name="xp", bufs=3) as xp,
            tc.tile_pool(name="qkp", bufs=2) as qkp,
            tc.tile_pool(name="vtp", bufs=3) as vtp,
            tc.tile_pool(name="outp", bufs=2) as outp,
            tc.tile_pool(name="ps_qk", bufs=2, space="PSUM") as ps_qk,
            tc.tile_pool(name="ps_vt", bufs=2, space="PSUM") as ps_vt,
            tc.tile_pool(name="ps_at", bufs=2, space="PSUM") as ps_at,
            tc.tile_pool(name="ps_out", bufs=2, space="PSUM") as ps_out,
        ):
            # ---- constants (SWDGE queue, keeping the sync queue free for
            # the first big x loads) ----
            wq_sb = const.tile([128, 2 * C8], F32R)
            nc.gpsimd.dma_start(wq_sb[:, 0:C8], wqt[0:128, :])
            nc.gpsimd.dma_start(wq_sb[:, C8:2 * C8], wqt[128:256, :])
            wk_sb = const.tile([128, 2 * C8], F32R)
            nc.gpsimd.dma_start(wk_sb[:, 0:C8], wkt[0:128, :])
            nc.gpsimd.dma_start(wk_sb[:, C8:2 * C8], wkt[128:256, :])
            wv_sb = const.tile([128, 2 * C], F32R)
            nc.gpsimd.dma_start(wv_sb[:, 0:C], wvt[0:128, :])
            nc.gpsimd.dma_start(wv_sb[:, C:2 * C], wvt[128:256, :])
            bq_sb = const.tile([C8, 1], F32)
            nc.gpsimd.dma_start(bq_sb[:], bq32[:])
            bk_sb = const.tile([C8, 1], F32)
            nc.gpsimd.dma_start(bk_sb[:], bk32[:])
            bv_row = const.tile([1, 2 * C], F32)
            nc.gpsimd.dma_start(bv_row[:], bvr2[:])

            # bv|bv broadcast to all 128 partitions: ones[1,128]^T @ bv_row
            ones = const.tile([1, 128], F32)
            nc.vector.memset(ones[:], 1.0)
            bv_bc = const.tile([128, 2 * C], F32)
            bv_ps = ps_vt.tile([128, 2 * C], F32, tag="vt")
            nc.tensor.matmul(bv_ps[:], ones[:], bv_row[:], start=True, stop=True)
            nc.vector.tensor_copy(bv_bc[:], bv_ps[:])

            # persistent attn tiles: off-diagonal quadrants stay zero forever,
            # only the in-pair diagonal quadrants are rewritten each use.
            at_tiles = []
            for t in range(3):
                at_t = const.tile([128, 128], F32R, tag=f"at{t}")
                nc.gpsimd.dma_start(at_t[:], atz[:])
                at_tiles.append(at_t)
            at_cnt = 0

            # ---- main loop, software-pipelined ----
            # q/k projections for block b+1 are interleaved into block b's
            # pair loop (one [32,512] n-tile group per couple) so the long
            # N=512 streams keep the PE array duty cycle high (HAM warm) and
            # q/k are ready a full block before the attention reads them.
            blocks = [(s, blk) for s in range(S) for blk in range(NBLK)]

            xts = {}

            def issue_x(bidx):
                s, blk = blocks[bidx]
                hw0 = blk * BLK
                # one 2 MiB DMA per input: tile cols = [ci0 2048 | ci1 2048]
                x1t = xp.tile([128, 2 * BLK], F32R, tag="x1t")
                nc.sync.dma_start(
                    x1t[:].rearrange("p (two c) -> p two c", two=2),
                    x1c[s, :, hw0:hw0 + BLK].rearrange("(two p) c -> p two c", p=128))
                x2t = xp.tile([128, 2 * BLK], F32R, tag="x2t")
                nc.sync.dma_start(
                    x2t[:].rearrange("p (two c) -> p two c", two=2),
                    x2c[s, :, hw0:hw0 + BLK].rearrange("(two p) c -> p two c", p=128))
                xts[bidx] = (x1t, x2t)

            def make_qk(bidx):
                q_sb = qkp.tile([C8, BLK], F32R, tag="q")
                k_sb = qkp.tile([C8, BLK], F32R, tag="k")
                return q_sb, k_sb

            def emit_qk_group(bidx, qk, g):
                # g in 0..7: even -> q n-tile, odd -> k n-tile; nt = g//2
                q_sb, k_sb = qk
                x1t, x2t = xts[bidx]
                nt = g // 2
                if g % 2 == 0:
                    w_sb, xt, dst, bias = wq_sb, x1t, q_sb, bq_sb
                else:
                    w_sb, xt, dst, bias = wk_sb, x2t, k_sb, bk_sb
                ps = ps_qk.tile([C8, 512], F32, tag="qk")
                for ci in range(2):
                    nc.tensor.matmul(
                        ps[:],
                        w_sb[:, ci * C8:(ci + 1) * C8],
                        xt[:, 2048 * ci + 512 * nt:2048 * ci + 512 * (nt + 1)],
                        start=(ci == 0), stop=(ci == 1),
                    )
                nc.scalar.activation(
                    dst[:, 512 * nt:512 * (nt + 1)], ps[:],
                    mybir.ActivationFunctionType.Identity, bias=bias[:])

            # prologue: block 0 loads + projections
            issue_x(0)
            qk_cur = make_qk(0)
            for g in range(8):
                emit_qk_group(0, qk_cur, g)

            for bidx in range(len(blocks)):
                s, blk = blocks[bidx]
                hw0 = blk * BLK
                x1t, x2t = xts[bidx]
                q_sb, k_sb = qk_cur
                has_next = bidx + 1 < len(blocks)
                if has_next:
                    issue_x(bidx + 1)
                    qk_next = make_qk(bidx + 1)

                outT = outp.tile([128, NPAIR * C], F32, tag="outT")

                for t in range(8):           # 2-pair couples
                    if has_next:
                        emit_qk_group(bidx + 1, qk_next, t)
                    # vT for both pairs into one [128,512] psum
                    vps = ps_vt.tile([128, 2 * C], F32, tag="vt")
                    for ii in range(2):
                        col = 128 * (2 * t + ii)
                        for ci in range(2):
                            nc.tensor.matmul(
                                vps[:, C * ii:C * (ii + 1)],
                                x2t[:, 2048 * ci + col:2048 * ci + col + 128],
                                wv_sb[:, ci * C:(ci + 1) * C],
                                start=(ci == 0), stop=(ci == 1),
                            )
                    vt_sb = vtp.tile([128, 2 * C], F32R, tag="vt_sb")
                    nc.vector.tensor_add(vt_sb[:], vps[:], bv_bc[:])

                    ops = ps_out.tile([128, 2 * C], F32, tag="po")
                    for ii in range(2):
                        p = 2 * t + ii       # h-pair index in block
                        col = 128 * p        # hw offset in block
                        # attn cross-block [y, x over the 2-pair window]
                        aps = ps_at.tile([128, 256], F32, tag="at")
                        nc.tensor.matmul(
                            aps[:],
                            k_sb[:, col:col + 128],
                            q_sb[:, 256 * t:256 * (t + 1)],
                            start=True, stop=True,
                        )
                        at_sb = at_tiles[at_cnt % 3]
                        at_cnt += 1
                        xo = 128 * ii        # this pair's x cols in aps
                        nc.vector.tensor_copy(at_sb[0:64, 0:64], aps[0:64, xo:xo + 64])
                        nc.vector.tensor_copy(at_sb[64:128, 64:128], aps[64:128, xo + 64:xo + 128])

                        # outT[x,c] = sum_y at[y,x] vt[y,c]
                        nc.tensor.matmul(
                            ops[:, C * ii:C * (ii + 1)],
                            at_sb[:], vt_sb[:, C * ii:C * (ii + 1)],
                            start=True, stop=True,
                        )
                    nc.scalar.activation(
                        outT[:, 2 * C * t:2 * C * (t + 1)], ops[:],
                        mybir.ActivationFunctionType.Copy,
                    )

                dst = outc[s, hw0:hw0 + BLK, :].rearrange("(p r) c -> r p c", r=128)
                src = outT[:].rearrange("r (p c) -> r p c", c=C)
                nc.sync.dma_start(dst, src)

                qk_cur = qk_next if has_next else None

    nc.compile()
    return nc


_NC_CACHE = None


def _get_nc():
    global _NC_CACHE
    if _NC_CACHE is None:
        _NC_CACHE = build_nc()
    return _NC_CACHE


def _make_in_maps(x1, x2, Wq, bq, Wk, bk, Wv, bv):
    x1 = np.asarray(x1, dtype=np.float32)
    x2 = np.asarray(x2, dtype=np.float32)
    x1s = np.ascontiguousarray(x1.transpose(0, 2, 1, 3, 4)).reshape(B * D, C, HWF)
    x2s = np.ascontiguousarray(x2.transpose(0, 2, 1, 3, 4)).reshape(B * D, C, HWF)
    wqt = np.ascontiguousarray(np.asarray(Wq, np.float32).T)
    wkt = np.ascontiguousarray(np.asarray(Wk, np.float32).T)
    wvt = np.ascontiguousarray(np.asarray(Wv, np.float32).T)
    bq32 = np.ascontiguousarray(np.asarray(bq, np.float32)[:, None])
    bk32 = np.ascontiguousarray(np.asarray(bk, np.float32)[:, None])
    bv = np.asarray(bv, np.float32)
    bvr2 = np.ascontiguousarray(np.concatenate([bv, bv])[None, :])
    in_maps = []
    for c in range(NCORES):
        in_maps.append(dict(
            x1c=np.ascontiguousarray(x1s[S * c:S * (c + 1)]),
            x2c=np.ascontiguousarray(x2s[S * c:S * (c + 1)]),
            wqt=wqt, wkt=wkt, wvt=wvt, bq32=bq32, bk32=bk32, bvr2=bvr2,
            atz=np.zeros((128, 128), np.float32),
        ))
    return in_maps


def _assemble(results):
    outs = np.concatenate([np.asarray(r["outc"]) for r in results], axis=0)
    # [32, 4096, 256] -> [B, D, H, W, C] -> [B, C, D, H, W]
    out = outs.reshape(B, D, H, W, C).transpose(0, 4, 1, 2, 3)
    return np.ascontiguousarray(out)


def kernel(x1, x2, Wq, bq, Wk, bk, Wv, bv, _trace=False):
    nc = _get_nc()
    in_maps = _make_in_maps(x1, x2, Wq, bq, Wk, bk, Wv, bv)
    res = run_bass_kernel_spmd(nc, in_maps, list(range(NCORES)), trace=_trace)
    out = _assemble(res.results)
    if _trace:
        return out, res
    return out


# revision 20
# speedup vs baseline: 1.1126x; 1.1126x over previous
"""Trainium2 Bass kernel for nn_CrossAttention (non-softmax bilinear attention).

Reference computation (B=2, C=256, C8=32, D=16, H=64, W=64):
    q = conv1x1(x1, Wq) + bq           [B,C8,D,H,W]
    k = conv1x1(x2, Wk) + bk           [B,C8,D,H,W]
    v = conv1x1(x2, Wv) + bv           [B,C ,D,H,W]
    attn[b,d,h,x,y] = sum_c q[b,c,d,h,x] k[b,c,d,h,y]
    out[b,c,d,h,x]  = sum_y attn[b,d,h,x,y] v[b,c,d,h,y]

Sharding: the (b,d) axis (32 slices) is split across 8 cores, 4 slices each.
Each core processes its slices in hw-blocks of 2048 (32 h rows = 16 h-pairs,
8 two-pair "couples"):
  - All matmuls run in float32r (1 cycle/row at N>=256, vs 4 for plain fp32).
    f32r constraints: producers of matmul inputs are DMAs, DVE copies/adds, or
    ACT Identity ops writing f32r-tagged tiles; N>=256; tile_position (0,0).
  - q,k are [32, 2048] tiles (channels on partitions 0:32), bias added by the
    scalar engine during PSUM eviction.
  - v is produced directly transposed (vT[y,c], +bv via a broadcast tile) by
    using the x2 block as the stationary operand; PSUM evictions are batched
    per couple ([128,512]) to halve DVE/ACT op count.
  - attention per h-pair as a [128,256] cross block over the couple's
    x-window; h-cross terms are zeroed on SBUF (3 persistent attn tiles whose
    off-diagonal quadrants are zeroed once and never rewritten).
  - out is computed transposed: outT[x,c] = at[y,x]^T @ vT[y,c], one N=256
    matmul per pair accumulating into a per-couple [128,512] PSUM tile; the
    output DRAM tensor is [S, HW, C] and the host transposes back.
"""

import sys
import numpy as np

if "/opt/trn_rl_repo" not in sys.path:
    sys.path.insert(0, "/opt/trn_rl_repo")

import concourse.bass as bass
import concourse.bacc as bacc
import concourse.mybir as mybir
from concourse import tile
from concourse.bass_utils import run_bass_kernel_spmd

F32 = mybir.dt.float32
F32R = mybir.dt.float32r

B, C, D, H, W = 2, 256, 16, 64, 64
C8 = C // 8
NCORES = 8
S = (B * D) // NCORES          # (b,d) slices per core
HWF = H * W                    # 4096
BLK = 2048                     # hw columns per block
NBLK = HWF // BLK
NPAIR = BLK // 128             # 16 h-pairs per block


def build_nc() -> bass.Bass:
    # Bacc (vs plain Bass) is required: its compile() pipeline runs
    # move_matmul_waits_to_ldweights + generate_event_semaphores, which
    # split multi-sem waits that the TRN2 Matmult encoding can't carry.
    nc = bacc.Bacc()

    x1c = nc.declare_dram_parameter("x1c", [S, C, HWF], F32R, isOutput=False)
    x2c = nc.declare_dram_parameter("x2c", [S, C, HWF], F32R, isOutput=False)
    wqt = nc.declare_dram_parameter("wqt", [C, C8], F32R, isOutput=False)
    wkt = nc.declare_dram_parameter("wkt", [C, C8], F32R, isOutput=False)
    wvt = nc.declare_dram_parameter("wvt", [C, C], F32R, isOutput=False)
    bq32 = nc.declare_dram_parameter("bq32", [C8, 1], F32, isOutput=False)
    bk32 = nc.declare_dram_parameter("bk32", [C8, 1], F32, isOutput=False)
    bvr2 = nc.declare_dram_parameter("bvr2", [1, 2 * C], F32, isOutput=False)
    atz = nc.declare_dram_parameter("atz", [128, 128], F32R, isOutput=False)
    outc = nc.declare_dram_parameter("outc", [S, HWF, C], F32, isOutput=True)

    with tile.TileContext(nc) as tc:
        with (
            tc.tile_pool(name="const", bufs=1) as const,
            tc.tile_pool(name="xp", bufs=3) as xp,
            tc.tile_pool(name="qkp", bufs=2) as qkp,
            tc.tile_pool(name="vtp", bufs=3) as vtp,
            tc.tile_pool(name="outp", bufs=2) as outp,
            tc.tile_pool(name="ps_qk", bufs=2, space="PSUM") as ps_qk,
            tc.tile_pool(name="ps_vt", bufs=2, space="PSUM") as ps_vt,
            tc.tile_pool(name="ps_at", bufs=2, space="PSUM") as ps_at,
            tc.tile_pool(name="ps_out", bufs=2, space="PSUM") as ps_out,
        ):
            # ---- constants (SWDGE queue, keeping the sync queue free for
            # the first big x loads) ----
            wq_sb = const.tile([128, 2 * C8], F32R)
            nc.gpsimd.dma_start(wq_sb[:, 0:C8], wqt[0:128, :])
            nc.gpsimd.dma_start(wq_sb[:, C8:2 * C8], wqt[128:256, :])
            wk_sb = const.tile([128, 2 * C8], F32R)
            nc.gpsimd.dma_start(wk_sb[:, 0:C8], wkt[0:128, :])
            nc.gpsimd.dma_start(wk_sb[:, C8:2 * C8], wkt[128:256, :])
            wv_sb = const.tile([128, 2 * C], F32R)
            nc.gpsimd.dma_start(wv_sb[:, 0:C], wvt[0:128, :])
            nc.gpsimd.dma_start(wv_sb[:, C:2 * C], wvt[128:256, :])
            bq_sb = const.tile([C8, 1], F32)
            nc.gpsimd.dma_start(bq_sb[:], bq32[:])
            bk_sb = const.tile([C8, 1], F32)
            nc.gpsimd.dma_start(bk_sb[:], bk32[:])
            bv_row = const.tile([1, 2 * C], F32)
            nc.gpsimd.dma_start(bv_row[:], bvr2[:])

            # bv|bv broadcast to all 128 partitions: ones[1,128]^T @ bv_row
            ones = const.tile([1, 128], F32)
            nc.vector.memset(ones[:], 1.0)
            bv_bc = const.tile([128, 2 * C], F32)
            bv_ps = ps_vt.tile([128, 2 * C], F32, tag="vt")
            nc.tensor.matmul(bv_ps[:], ones[:], bv_row[:], start=True, stop=True)
            nc.vector.tensor_copy(bv_bc[:], bv_ps[:])

            # persistent attn tiles: off-diagonal quadrants stay zero forever,
            # only the in-pair diagonal quadrants are rewritten each use.
            at_tiles = []
            for t in range(4):
                at_t = const.tile([128, 128], F32R, tag=f"at{t}")
                nc.gpsimd.dma_start(at_t[:], atz[:])
                at_tiles.append(at_t)
            at_cnt = 0

            # ---- main loop, software-pipelined ----
            # q/k projections for block b+1 are interleaved into block b's
            # pair loop (one [32,512] n-tile group per couple) so the long
            # N=512 streams keep the PE array duty cycle high (HAM warm) and
            # q/k are ready a full block before the attention reads them.
            blocks = [(s, blk) for s in range(S) for blk in range(NBLK)]

            xts = {}

            def issue_x(bidx):
                s, blk = blocks[bidx]
                hw0 = blk * BLK
                # one 2 MiB DMA per input: tile cols = [ci0 2048 | ci1 2048]
                x1t = xp.tile([128, 2 * BLK], F32R, tag="x1t")
                nc.sync.dma_start(
                    x1t[:].rearrange("p (two c) -> p two c", two=2),
                    x1c[s, :, hw0:hw0 + BLK].rearrange("(two p) c -> p two c", p=128))
                x2t = xp.tile([128, 2 * BLK], F32R, tag="x2t")
                nc.sync.dma_start(
                    x2t[:].rearrange("p (two c) -> p two c", two=2),
                    x2c[s, :, hw0:hw0 + BLK].rearrange("(two p) c -> p two c", p=128))
                xts[bidx] = (x1t, x2t)

            def make_qk(bidx):
                q_sb = qkp.tile([C8, BLK], F32R, tag="q")
                k_sb = qkp.tile([C8, BLK], F32R, tag="k")
                return q_sb, k_sb

            def emit_qk_group(bidx, qk, g):
                # g in 0..7: even -> q n-tile, odd -> k n-tile; nt = g//2
                q_sb, k_sb = qk
                x1t, x2t = xts[bidx]
                nt = g // 2
                if g % 2 == 0:
                    w_sb, xt, dst, bias = wq_sb, x1t, q_sb, bq_sb
                else:
                    w_sb, xt, dst, bias = wk_sb, x2t, k_sb, bk_sb
                ps = ps_qk.tile([C8, 512], F32, tag="qk")
                for ci in range(2):
                    nc.tensor.matmul(
                        ps[:],
                        w_sb[:, ci * C8:(ci + 1) * C8],
                        xt[:, 2048 * ci + 512 * nt:2048 * ci + 512 * (nt + 1)],
                        start=(ci == 0), stop=(ci == 1),
                    )
                nc.scalar.activation(
                    dst[:, 512 * nt:512 * (nt + 1)], ps[:],
                    mybir.ActivationFunctionType.Identity, bias=bias[:])

            # prologue: block 0 loads + projections
            issue_x(0)
            qk_cur = make_qk(0)
            for g in range(8):
                emit_qk_group(0, qk_cur, g)

            for bidx in range(len(blocks)):
                s, blk = blocks[bidx]
                hw0 = blk * BLK
                x1t, x2t = xts[bidx]
                q_sb, k_sb = qk_cur
                has_next = bidx + 1 < len(blocks)
                if has_next:
                    issue_x(bidx + 1)
                    qk_next = make_qk(bidx + 1)

                outT = outp.tile([128, NPAIR * C], F32, tag="outT")

                for t in range(8):           # 2-pair couples
                    if has_next:
                        emit_qk_group(bidx + 1, qk_next, t)
                    # attention first: gives the DVE a head start on the
                    # diagonal-quadrant copies the out matmuls depend on.
                    pair_at = []
                    for ii in range(2):
                        col = 128 * (2 * t + ii)
                        aps = ps_at.tile([128, 256], F32, tag="at")
                        nc.tensor.matmul(
                            aps[:],
                            k_sb[:, col:col + 128],
                            q_sb[:, 256 * t:256 * (t + 1)],
                            start=True, stop=True,
                        )
                        at_sb = at_tiles[at_cnt % 4]
                        at_cnt += 1
                        xo = 128 * ii        # this pair's x cols in aps
                        nc.vector.tensor_copy(at_sb[0:64, 0:64], aps[0:64, xo:xo + 64])
                        nc.vector.tensor_copy(at_sb[64:128, 64:128], aps[64:128, xo + 64:xo + 128])
                        pair_at.append(at_sb)

                    # vT for both pairs into one [128,512] psum
                    vps = ps_vt.tile([128, 2 * C], F32, tag="vt")
                    for ii in range(2):
                        col = 128 * (2 * t + ii)
                        for ci in range(2):
                            nc.tensor.matmul(
                                vps[:, C * ii:C * (ii + 1)],
                                x2t[:, 2048 * ci + col:2048 * ci + col + 128],
                                wv_sb[:, ci * C:(ci + 1) * C],
                                start=(ci == 0), stop=(ci == 1),
                            )
                    vt_sb = vtp.tile([128, 2 * C], F32R, tag="vt_sb")
                    nc.vector.tensor_add(vt_sb[:], vps[:], bv_bc[:])

                    ops = ps_out.tile([128, 2 * C], F32, tag="po")
                    for ii in range(2):
                        # outT[x,c] = sum_y at[y,x] vt[y,c]
                        nc.tensor.matmul(
                            ops[:, C * ii:C * (ii + 1)],
                            pair_at[ii][:], vt_sb[:, C * ii:C * (ii + 1)],
                            start=True, stop=True,
                        )
                    nc.scalar.activation(
                        outT[:, 2 * C * t:2 * C * (t + 1)], ops[:],
                        mybir.ActivationFunctionType.Copy,
                    )

                dst = outc[s, hw0:hw0 + BLK, :].rearrange("(p r) c -> r p c", r=128)
                src = outT[:].rearrange("r (p c) -> r p c", c=C)
                nc.sync.dma_start(dst, src)

                qk_cur = qk_next if has_next else None

    nc.compile()
    return nc


_NC_CACHE = None


def _get_nc():
    global _NC_CACHE
    if _NC_CACHE is None:
        _NC_CACHE = build_nc()
    return _NC_CACHE


def _make_in_maps(x1, x2, Wq, bq, Wk, bk, Wv, bv):
    x1 = np.asarray(x1, dtype=np.float32)
    x2 = np.asarray(x2, dtype=np.float32)
    x1s = np.ascontiguousarray(x1.transpose(0, 2, 1, 3, 4)).reshape(B * D, C, HWF)
    x2s = np.ascontiguousarray(x2.transpose(0, 2, 1, 3, 4)).reshape(B * D, C, HWF)
    wqt = np.ascontiguousarray(np.asarray(Wq, np.float32).T)
    wkt = np.ascontiguousarray(np.asarray(Wk, np.float32).T)
    wvt = np.ascontiguousarray(np.asarray(Wv, np.float32).T)
    bq32 = np.ascontiguousarray(np.asarray(bq, np.float32)[:, None])
    bk32 = np.ascontiguousarray(np.asarray(bk, np.float32)[:, None])
    bv = np.asarray(bv, np.float32)
    bvr2 = np.ascontiguousarray(np.concatenate([bv, bv])[None, :])
    in_maps = []
    for c in range(NCORES):
        in_maps.append(dict(
            x1c=np.ascontiguousarray(x1s[S * c:S * (c + 1)]),
            x2c=np.ascontiguousarray(x2s[S * c:S * (c + 1)]),
            wqt=wqt, wkt=wkt, wvt=wvt, bq32=bq32, bk32=bk32, bvr2=bvr2,
            atz=np.zeros((128, 128), np.float32),
        ))
    return in_maps


def _assemble(results):
    outs = np.concatenate([np.asarray(r["outc"]) for r in results], axis=0)
    # [32, 4096, 256] -> [B, D, H, W, C] -> [B, C, D, H, W]
    out = outs.reshape(B, D, H, W, C).transpose(0, 4, 1, 2, 3)
    return np.ascontiguousarray(out)


def kernel(x1, x2, Wq, bq, Wk, bk, Wv, bv, _trace=False):
    nc = _get_nc()
    in_maps = _make_in_maps(x1, x2, Wq, bq, Wk, bk, Wv, bv)
    res = run_bass_kernel_spmd(nc, in_maps, list(range(NCORES)), trace=_trace)
    out = _assemble(res.results)
    if _trace:
        return out, res
    return out


# revision 25
# speedup vs baseline: 1.3483x; 1.2119x over previous
"""Trainium2 Bass kernel for nn_CrossAttention (non-softmax bilinear attention).

Reference computation (B=2, C=256, C8=32, D=16, H=64, W=64):
    q = conv1x1(x1, Wq) + bq           [B,C8,D,H,W]
    k = conv1x1(x2, Wk) + bk           [B,C8,D,H,W]
    v = conv1x1(x2, Wv) + bv           [B,C ,D,H,W]
    attn[b,d,h,x,y] = sum_c q[b,c,d,h,x] k[b,c,d,h,y]
    out[b,c,d,h,x]  = sum_y attn[b,d,h,x,y] v[b,c,d,h,y]

Sharding: the (b,d) axis (32 slices) is split across 8 cores, 4 slices each.
Each core processes its slices in hw-blocks of 2048 (32 h rows = 16 h-pairs,
8 two-pair "couples"):
  - All matmuls run in float32r (1 cycle/row at N>=256, vs 4 for plain fp32).
    f32r constraints: producers of matmul inputs are DMAs, DVE copies/adds, or
    ACT Identity ops writing f32r-tagged tiles; N>=256; tile_position (0,0).
  - q,k are [32, 2048] tiles (channels on partitions 0:32), bias added by the
    scalar engine during PSUM eviction.
  - v is produced directly transposed (vT[y,c], +bv via a broadcast tile) by
    using the x2 block as the stationary operand; PSUM evictions are batched
    per couple ([128,512]) to halve DVE/ACT op count.
  - attention per h-pair as a [128,256] cross block over the couple's
    x-window; h-cross terms are zeroed on SBUF (3 persistent attn tiles whose
    off-diagonal quadrants are zeroed once and never rewritten).
  - out is computed transposed: outT[x,c] = at[y,x]^T @ vT[y,c], one N=256
    matmul per pair accumulating into a per-couple [128,512] PSUM tile; the
    output DRAM tensor is [S, HW, C] and the host transposes back.
"""

import sys
import numpy as np

if "/opt/trn_rl_repo" not in sys.path:
    sys.path.insert(0, "/opt/trn_rl_repo")

import concourse.bass as bass
import concourse.bacc as bacc
import concourse.mybir as mybir
from concourse import tile
from concourse.bass_utils import run_bass_kernel_spmd

F32 = mybir.dt.float32
F32R = mybir.dt.float32r

B, C, D, H, W = 2, 256, 16, 64, 64
C8 = C // 8
NCORES = 8
S = (B * D) // NCORES          # (b,d) slices per core
HWF = H * W                    # 4096
BLK = 2048                     # hw columns per block
NBLK = HWF // BLK
NPAIR = BLK // 128             # 16 h-pairs per block


def build_nc() -> bass.Bass:
    # Bacc (vs plain Bass) is required: its compile() pipeline runs
    # move_matmul_waits_to_ldweights + generate_event_semaphores, which
    # split multi-sem waits that the TRN2 Matmult encoding can't carry.
    nc = bacc.Bacc()

    # all f32r constants packed host-side into one tensor -> one DMA
    # cols: [wq 0:64 | wk 64:128 | wv 128:640 | zeros 640:768]
    wpack = nc.declare_dram_parameter("wpack", [128, 768], F32R, isOutput=False)
    bpack = nc.declare_dram_parameter("bpack", [128, 2], F32, isOutput=False)
    bvr2 = nc.declare_dram_parameter("bvr2", [1, 2 * C], F32, isOutput=False)
    x1c = nc.declare_dram_parameter("x1c", [S, C, HWF], F32R, isOutput=False)
    x2c = nc.declare_dram_parameter("x2c", [S, C, HWF], F32R, isOutput=False)
    outc = nc.declare_dram_parameter("outc", [S, HWF, C], F32, isOutput=True)

    with tile.TileContext(nc) as tc:
        with (
            tc.tile_pool(name="const", bufs=1) as const,
            tc.tile_pool(name="xp", bufs=3) as xp,
            tc.tile_pool(name="qkp", bufs=2) as qkp,
            tc.tile_pool(name="vtp", bufs=3) as vtp,
            tc.tile_pool(name="outp", bufs=2) as outp,
            tc.tile_pool(name="ps_qk", bufs=2, space="PSUM") as ps_qk,
            tc.tile_pool(name="ps_vt", bufs=2, space="PSUM") as ps_vt,
            tc.tile_pool(name="ps_at", bufs=2, space="PSUM") as ps_at,
            tc.tile_pool(name="ps_out", bufs=2, space="PSUM") as ps_out,
        ):
            # ---- constants: 3 small HWDGE DMAs issued before the x loads ----
            wp_sb = const.tile([128, 768], F32R)
            nc.sync.dma_start(wp_sb[:], wpack[:])
            bp_sb = const.tile([128, 2], F32)
            nc.sync.dma_start(bp_sb[:], bpack[:])
            bv_row = const.tile([1, 2 * C], F32)
            nc.sync.dma_start(bv_row[:], bvr2[:])

            wq_sb = wp_sb[:, 0:64]
            wk_sb = wp_sb[:, 64:128]
            wv_sb = wp_sb[:, 128:640]
            zz_sb = wp_sb[:, 640:768]
            bq_sb = bp_sb[0:C8, 0:1]
            bk_sb = bp_sb[0:C8, 1:2]

            # bv|bv broadcast to all 128 partitions: ones[1,128]^T @ bv_row
            ones = const.tile([1, 128], F32)
            nc.vector.memset(ones[:], 1.0)
            bv_bc = const.tile([128, 2 * C], F32)
            bv_ps = ps_vt.tile([128, 2 * C], F32, tag="vt")
            nc.tensor.matmul(bv_ps[:], ones[:], bv_row[:], start=True, stop=True)
            nc.vector.tensor_copy(bv_bc[:], bv_ps[:])

            # persistent attn tiles: off-diagonal quadrants stay zero forever,
            # only the in-pair diagonal quadrants are rewritten each use.
            at_tiles = []
            for t in range(4):
                at_t = const.tile([128, 128], F32R, tag=f"at{t}")
                nc.vector.tensor_copy(at_t[:], zz_sb[:])
                at_tiles.append(at_t)
            at_cnt = 0

            # ---- main loop, software-pipelined ----
            # q/k projections for block b+1 are interleaved into block b's
            # pair loop (one [32,512] n-tile group per couple) so the long
            # N=512 streams keep the PE array duty cycle high (HAM warm) and
            # q/k are ready a full block before the attention reads them.
            blocks = [(s, blk) for s in range(S) for blk in range(NBLK)]

            xts = {}

            def issue_x(bidx):
                s, blk = blocks[bidx]
                hw0 = blk * BLK
                # one 2 MiB DMA per input: tile cols = [ci0 2048 | ci1 2048]
                x1t = xp.tile([128, 2 * BLK], F32R, tag="x1t")
                nc.sync.dma_start(
                    x1t[:].rearrange("p (two c) -> p two c", two=2),
                    x1c[s, :, hw0:hw0 + BLK].rearrange("(two p) c -> p two c", p=128))
                x2t = xp.tile([128, 2 * BLK], F32R, tag="x2t")
                nc.sync.dma_start(
                    x2t[:].rearrange("p (two c) -> p two c", two=2),
                    x2c[s, :, hw0:hw0 + BLK].rearrange("(two p) c -> p two c", p=128))
                xts[bidx] = (x1t, x2t)

            def make_qk(bidx):
                q_sb = qkp.tile([C8, BLK], F32R, tag="q")
                k_sb = qkp.tile([C8, BLK], F32R, tag="k")
                return q_sb, k_sb

            def emit_qk_group(bidx, qk, g):
                # g in 0..7: even -> q n-tile, odd -> k n-tile; nt = g//2
                q_sb, k_sb = qk
                x1t, x2t = xts[bidx]
                nt = g // 2
                if g % 2 == 0:
                    w_sb, xt, dst, bias = wq_sb, x1t, q_sb, bq_sb
                else:
                    w_sb, xt, dst, bias = wk_sb, x2t, k_sb, bk_sb
                ps = ps_qk.tile([C8, 512], F32, tag="qk")
                for ci in range(2):
                    nc.tensor.matmul(
                        ps[:],
                        w_sb[:, ci * C8:(ci + 1) * C8],
                        xt[:, 2048 * ci + 512 * nt:2048 * ci + 512 * (nt + 1)],
                        start=(ci == 0), stop=(ci == 1),
                    )
                nc.scalar.activation(
                    dst[:, 512 * nt:512 * (nt + 1)], ps[:],
                    mybir.ActivationFunctionType.Identity, bias=bias[:])

            # prologue: block 0 loads + projections (q groups first — x1
            # arrives before x2 on the FIFO queue)
            issue_x(0)
            issue_x(1)
            qk_cur = make_qk(0)
            for g in (0, 2, 4, 6, 1, 3, 5, 7):
                emit_qk_group(0, qk_cur, g)

            for bidx in range(len(blocks)):
                s, blk = blocks[bidx]
                hw0 = blk * BLK
                x1t, x2t = xts[bidx]
                q_sb, k_sb = qk_cur
                has_next = bidx + 1 < len(blocks)
                if bidx + 2 < len(blocks):
                    issue_x(bidx + 2)
                if has_next:
                    qk_next = make_qk(bidx + 1)

                outT = outp.tile([128, NPAIR * C], F32, tag="outT")

                for t in range(8):           # 2-pair couples
                    if has_next:
                        emit_qk_group(bidx + 1, qk_next, t)
                    # attention first: gives the DVE a head start on the
                    # diagonal-quadrant copies the out matmuls depend on.
                    pair_at = []
                    for ii in range(2):
                        col = 128 * (2 * t + ii)
                        aps = ps_at.tile([128, 256], F32, tag="at")
                        nc.tensor.matmul(
                            aps[:],
                            k_sb[:, col:col + 128],
                            q_sb[:, 256 * t:256 * (t + 1)],
                            start=True, stop=True,
                        )
                        at_sb = at_tiles[at_cnt % 4]
                        at_cnt += 1
                        xo = 128 * ii        # this pair's x cols in aps
                        nc.vector.tensor_copy(at_sb[0:64, 0:64], aps[0:64, xo:xo + 64])
                        nc.vector.tensor_copy(at_sb[64:128, 64:128], aps[64:128, xo + 64:xo + 128])
                        pair_at.append(at_sb)

                    # vT for both pairs into one [128,512] psum
                    vps = ps_vt.tile([128, 2 * C], F32, tag="vt")
                    for ii in range(2):
                        col = 128 * (2 * t + ii)
                        for ci in range(2):
                            nc.tensor.matmul(
                                vps[:, C * ii:C * (ii + 1)],
                                x2t[:, 2048 * ci + col:2048 * ci + col + 128],
                                wv_sb[:, ci * C:(ci + 1) * C],
                                start=(ci == 0), stop=(ci == 1),
                            )
                    vt_sb = vtp.tile([128, 2 * C], F32R, tag="vt_sb")
                    nc.vector.tensor_add(vt_sb[:], vps[:], bv_bc[:])

                    ops = ps_out.tile([128, 2 * C], F32, tag="po")
                    for ii in range(2):
                        # outT[x,c] = sum_y at[y,x] vt[y,c]
                        nc.tensor.matmul(
                            ops[:, C * ii:C * (ii + 1)],
                            pair_at[ii][:], vt_sb[:, C * ii:C * (ii + 1)],
                            start=True, stop=True,
                        )
                    nc.scalar.activation(
                        outT[:, 2 * C * t:2 * C * (t + 1)], ops[:],
                        mybir.ActivationFunctionType.Copy,
                    )

                    if t == 3 or t == 7:
                        # store half the block as soon as its couples finish
                        hb = (t - 3) // 4
                        cols = slice(BLK // 2 * hb, BLK // 2 * (hb + 1))
                        dst = outc[s, hw0:hw0 + BLK, :][cols, :].rearrange(
                            "(p r) c -> r p c", r=128)
                        src = outT[:, 8 * C * hb:8 * C * (hb + 1)].rearrange(
                            "r (p c) -> r p c", c=C)
                        nc.sync.dma_start(dst, src)

                qk_cur = qk_next if has_next else None

    nc.compile()
    return nc


_NC_CACHE = None


def _get_nc():
    global _NC_CACHE
    if _NC_CACHE is None:
        _NC_CACHE = build_nc()
    return _NC_CACHE


def _make_in_maps(x1, x2, Wq, bq, Wk, bk, Wv, bv):
    x1 = np.asarray(x1, dtype=np.float32)
    x2 = np.asarray(x2, dtype=np.float32)
    x1s = np.ascontiguousarray(x1.transpose(0, 2, 1, 3, 4)).reshape(B * D, C, HWF)
    x2s = np.ascontiguousarray(x2.transpose(0, 2, 1, 3, 4)).reshape(B * D, C, HWF)
    wqt = np.asarray(Wq, np.float32).T
    wkt = np.asarray(Wk, np.float32).T
    wvt = np.asarray(Wv, np.float32).T
    bq = np.asarray(bq, np.float32)
    bk = np.asarray(bk, np.float32)
    bv = np.asarray(bv, np.float32)
    wpack = np.zeros((128, 768), np.float32)
    wpack[:, 0:32] = wqt[0:128]
    wpack[:, 32:64] = wqt[128:256]
    wpack[:, 64:96] = wkt[0:128]
    wpack[:, 96:128] = wkt[128:256]
    wpack[:, 128:384] = wvt[0:128]
    wpack[:, 384:640] = wvt[128:256]
    bpack = np.zeros((128, 2), np.float32)
    bpack[0:C8, 0] = bq
    bpack[0:C8, 1] = bk
    bvr2 = np.ascontiguousarray(np.concatenate([bv, bv])[None, :])
    in_maps = []
    for c in range(NCORES):
        in_maps.append(dict(
            x1c=np.ascontiguousarray(x1s[S * c:S * (c + 1)]),
            x2c=np.ascontiguousarray(x2s[S * c:S * (c + 1)]),
            wpack=wpack, bpack=bpack, bvr2=bvr2,
        ))
    return in_maps


def _assemble(results):
    outs = np.concatenate([np.asarray(r["outc"]) for r in results], axis=0)
    # [32, 4096, 256] -> [B, D, H, W, C] -> [B, C, D, H, W]
    out = outs.reshape(B, D, H, W, C).transpose(0, 4, 1, 2, 3)
    return np.ascontiguousarray(out)


def kernel(x1, x2, Wq, bq, Wk, bk, Wv, bv, _trace=False):
    nc = _get_nc()
    in_maps = _make_in_maps(x1, x2, Wq, bq, Wk, bk, Wv, bv)
    res = run_bass_kernel_spmd(nc, in_maps, list(range(NCORES)), trace=_trace)
    out = _assemble(res.results)
    if _trace:
        return out, res
    return out
